# revision 38
# baseline (speedup 1.0000x reference)
"""Trainium2 Bass kernel for MultiHeadDilatedAttention.

Full inputs in, full output out. Sharding: 8 cores = (batch b in 0..3) x
(segment-position half). Each (b, s) pair is an independent attention problem
(attention runs across segments n at fixed position-in-segment s), so each
core handles b = c//2 and 64 of the 128 s values. No collectives needed: the
output rows t = s*64 + dil*l for a core's s-range form a contiguous chunk of
y[b].

v3 layout (all matmuls bf16 with fp32 PSUM accumulation):
  x cast to bf16 + transposed on host -> [ec, blk, 128, 1024] in DRAM
  pipelined per 16-segment block: DMA x block, project Q^T/K^T/V^T
  Q/K/V stored s-major fully packed (slot stride = L per head, no padding)
  V^T -> PE-transpose -> V natural [slot, dv] per 128-col group
  attention phase is s-block-major (16 s per block, all 4 heads), 2-deep
  software pipeline; masking is a 0/1 bf16 multiply on GpSimd after the
  Scalar exp (no PE mask-init matmuls); out-projection units for block sb
  are interleaved into block sb+1's attention steps so the PE never idles
  during the softmax chain. y stored bf16, upcast + b_out added on host.
"""

from contextlib import ExitStack

import numpy as np
import ml_dtypes

import concourse.bass as bass
import concourse.mybir as mybir
import concourse.tile as tile
from concourse import bacc
from concourse.bass_utils import run_bass_kernel_spmd

F32 = mybir.dt.float32
BF16 = mybir.dt.bfloat16
AX = mybir.AxisListType
EXP = mybir.ActivationFunctionType.Exp

B, T, E = 4, 8192, 1024
SEG = 128          # segment size (= #s positions overall)
NB = T // SEG      # 64 segments
NS = 64            # s values per core
ROWS = NB * NS     # 4096 rows per core
DK = 128
H = 4
DILS = [1, 2, 4, 8]
LS = [NB // d for d in DILS]       # [64, 32, 16, 8] attention length per head
G = [128 // l for l in LS]         # s-slots per 128 partitions: [2, 4, 8, 16]
NGRP = [NS // g for g in G]        # 128-col groups per head: [32, 16, 8, 4]
TW = [4, 4, 2, 1]                  # groups per attention tile (tile = 16 or 8 s)
NORM = float(1.0 / np.sqrt(DK))
NECHUNK = E // 128                 # 8
NBLK = 4                           # x pipeline blocks
BLKN = NB // NBLK                  # 16 segments per block
SBLK = 4                           # attention s-blocks (16 s each)
SBW = NS // SBLK                   # 16
MQOFF = [0, 512, 1024, 1280]       # mask col offset per head (stacked masks)
MTOT = 1408                        # sum of TW[h]*128
NEG = -1.0e10

# out-projection classes: offsets o in [0,64) grouped by which heads hit them.
def _classes():
    out = []
    o_all = list(range(64))
    out.append(([o for o in o_all if o % 2 == 1], [0]))           # odd
    out.append(([o for o in o_all if o % 4 == 2], [0, 1]))        # 2 mod 4
    out.append(([o for o in o_all if o % 8 == 4], [0, 1, 2]))     # 4 mod 8
    out.append(([o for o in o_all if o % 8 == 0], [0, 1, 2, 3]))  # 0 mod 8
    return out


CLASSES = _classes()
# atT[h] column layout: per-class blocks (so out-proj lhsT slices are
# contiguous). HEAD_BLOCKS[h] = [(cid, l_list)], HEAD_OFF[h][cid] = col offset.
HEAD_BLOCKS = {}
HEAD_OFF = {}
for _h in range(H):
    blocks, offs, off = [], {}, 0
    for _cid, (_ol, _heads) in enumerate(CLASSES):
        if _h in _heads:
            ll = [o // DILS[_h] for o in _ol]
            blocks.append((_cid, ll))
            offs[_cid] = off
            off += len(ll) * NS
    HEAD_BLOCKS[_h] = blocks
    HEAD_OFF[_h] = offs
    assert off == LS[_h] * NS


def build_program(max_phase: int = 5) -> bass.Bass:
    nc = bacc.Bacc("TRN2", target_bir_lowering=False, debug=False)
    xs = nc.dram_tensor("xs", [NECHUNK, NBLK, 128, BLKN * NS], BF16,
                        kind="ExternalInput").ap()
    wqkv = nc.dram_tensor("wqkv", [12, 128, NECHUNK * 128], BF16,
                          kind="ExternalInput").ap()
    wout = nc.dram_tensor("wout", [128, H * E], BF16, kind="ExternalInput").ap()
    maskd = nc.dram_tensor("masks", [128, MTOT], BF16,
                           kind="ExternalInput").ap()
    y = nc.dram_tensor("y", [ROWS, E], BF16, kind="ExternalOutput").ap()
    _build_phases(nc, max_phase, xs, wqkv, wout, maskd, y)
    nc.finalize()
    return nc


def _build_phases(nc, max_phase, xs, wqkv, wout, maskd, y):
    # round-robin engine copy (PSUM->SBUF traffic spread over DVE + Act;
    # GPSIMD cannot access PSUM)
    state = {"i": 0}
    _RR = (0, 0, 0, 1, 1)  # DVE is ~1.6x faster per copy: give it 60%

    def rr_copy(out_ap, in_ap):
        k = _RR[state["i"] % len(_RR)]
        state["i"] += 1
        if k == 0:
            nc.vector.tensor_copy(out=out_ap, in_=in_ap)
        else:
            nc.scalar.copy(out_ap, in_ap)

    with ExitStack() as ctx:
        tc = ctx.enter_context(tile.TileContext(nc))

        persist = ctx.enter_context(tc.tile_pool(name="persist", bufs=1))
        ident = persist.tile([128, 128], BF16, tag="ident")
        # identity built before any dma_start posts: affine_select lives on
        # gpsimd, which also issues DMA descriptors — ident must come first
        # so the PE warm-up transposes can start at ~0.3us
        nc.gpsimd.memset(ident, 0.0)
        nc.gpsimd.affine_select(
            out=ident, in_=ident, compare_op=mybir.AluOpType.not_equal,
            fill=1.0, base=0, pattern=[[-1, 128]], channel_multiplier=1)

        # all inputs staged up front (x fully resident: 8 MiB of SBUF);
        # descriptor posts round-robin over sync/scalar/gpsimd, ordered so
        # the h0 weights + x block 0 land first
        w_sb = {}
        for h in range(H):
            for p in range(3):
                w_sb[(h, p)] = persist.tile([128, NECHUNK * 128], BF16,
                                            tag=f"w{h}{p}", name=f"w{h}{p}")
        xt_pool = ctx.enter_context(tc.tile_pool(name="xt", bufs=1))
        xt = {}
        for blk in range(NBLK):
            for ec in range(NECHUNK):
                xt[(blk, ec)] = xt_pool.tile(
                    [128, BLKN * NS], BF16, tag=f"x{blk}_{ec}",
                    name=f"x{blk}_{ec}")
        wout_sb = persist.tile([128, H * E], BF16, tag="wout_sb")
        mask_sb = persist.tile([128, MTOT], BF16, tag="mask_sb")

        # x tiles load in two 512-col halves; all blocks' first halves load
        # first so h0's l<8 projection tiles (which only read cols 0:512)
        # unlock ~21us of PE work before the second halves arrive — this
        # hides the x-DMA latency (DMA shares bandwidth with i-fetch)
        posts = [(w_sb[(0, 0)], wqkv[0, :, :])]
        for blk in range(NBLK):
            for ec in range(NECHUNK):
                posts.append((xt[(blk, ec)], xs[ec, blk, :, :]))
            if blk == 0:
                posts.append((w_sb[(0, 1)], wqkv[1, :, :]))
                posts.append((w_sb[(0, 2)], wqkv[2, :, :]))
        for h in (1, 2, 3):
            for p in range(3):
                posts.append((w_sb[(h, p)], wqkv[h * 3 + p, :, :]))
        posts.append((mask_sb, maskd))
        posts.append((wout_sb, wout))
        dma_engs = [nc.sync, nc.scalar, nc.gpsimd]
        for i, (dst, src) in enumerate(posts):
            dma_engs[i % 3].dma_start(out=dst, in_=src)

        # warm the PE clock gate (HAM) during the initial DMA wait: ~3.5us of
        # dummy transposes flips the PE to 2.4 GHz before real work arrives
        with ExitStack() as wctx:
            warm_ps = wctx.enter_context(
                tc.tile_pool(name="warm_ps", bufs=1, space="PSUM"))
            wps = warm_ps.tile([128, 128], BF16, tag="warm")
            for _ in range(36):
                nc.tensor.transpose(wps, ident, ident)

        # persistent per-head tensors: Q^T/K^T/V^T all stored s-major fully
        # packed (col = s*L + l), so a slot-group slice is one contiguous
        # 128-col window with no dead slots.
        qkvpool = ctx.enter_context(tc.tile_pool(name="qkv", bufs=1))
        qkv_sb = {}
        for h in range(H):
            for p in range(3):
                qkv_sb[(h, p)] = qkvpool.tile([128, LS[h] * NS], BF16,
                                              tag=f"qkv{h}{p}", name=f"qkv{h}{p}")
        atpool = ctx.enter_context(tc.tile_pool(name="atT", bufs=1))
        # plain s-major layout (col = s*L + l): the att scatter is then ONE
        # contiguous copy per tile; the out-proj reads strided lhsT slices
        # instead (stride cost moves to LDWEIGHTS, off the copy engines)
        atT = [atpool.tile([128, LS[h] * NS], BF16, tag=f"atT{h}",
                           name=f"atT{h}") for h in range(H)]
        vnpool = ctx.enter_context(tc.tile_pool(name="vnat", bufs=1))
        # vnat[h]: [128, NGRP, 128]; group gi holds V for s = gi*G[h]..+G[h]
        # at partition slots (s % G[h]) * L[h] + n
        vnat = [vnpool.tile([128, NGRP[h], 128], BF16, tag=f"vnat{h}",
                            name=f"vnat{h}") for h in range(H)]

        # one PSUM pool, tag = bank budget: pj 2 + kq 2 + at 1 + y 3 = 8
        psum = ctx.enter_context(tc.tile_pool(name="psum", bufs=1,
                                              space="PSUM"))
        sm_pool = ctx.enter_context(tc.tile_pool(name="sm", bufs=3))
        small = ctx.enter_context(tc.tile_pool(name="small", bufs=4))
        yo_pool = ctx.enter_context(tc.tile_pool(name="y_sb", bufs=3))

        # ---------------- emission closures -------------------------------
        def proj_tile(h, p, blk, lt, lcnt):
            def emit():
                dil = DILS[h]
                tcols = lcnt * NS
                l0 = blk * (BLKN // dil)
                ps = psum.tile([128, 512], F32, tag="pj", bufs=2,
                               name="pjps")[:, :tcols]
                for ec in range(NECHUNK):
                    lhsT = w_sb[(h, p)][:, ec * 128:(ec + 1) * 128]
                    rhs = xt[(blk, ec)].rearrange(
                        "p (l j s) -> p l j s", j=dil, s=NS
                    )[:, lt:lt + lcnt, 0, :]
                    nc.tensor.matmul(ps, lhsT, rhs, start=(ec == 0),
                                     stop=(ec == NECHUNK - 1))
                out_ap = qkv_sb[(h, p)].rearrange(
                    "p (s l) -> p s l", l=LS[h]
                )[:, :, l0 + lt:l0 + lt + lcnt]
                rr_copy(out_ap, ps.rearrange("p (l s) -> p s l", s=NS))
            return emit

        def proj_feed_blk(h, blk):
            out = []
            ln = BLKN // DILS[h]
            for p in range(3):
                for lt in range(0, ln, 8):
                    out.append(proj_tile(h, p, blk, lt, min(8, ln - lt)))
            return out

        def vt_batch(h, gi):
            def emit():
                # 4 transposes into one psum tile -> one wide PSUM->SBUF copy
                vt = qkv_sb[(h, 2)]
                pt = psum.tile([128, 512], BF16, tag="pj", bufs=2, name="vtps")
                for q in range(4):
                    nc.tensor.transpose(
                        pt[:, q * 128:(q + 1) * 128],
                        vt[:, (gi + q) * 128:(gi + q + 1) * 128], ident)
                rr_copy(vnat[h][:, gi:gi + 4, :], pt)
            return emit

        def vt_feed(h):
            return [vt_batch(h, gi) for gi in range(0, NGRP[h], 4)]

        def emit_kq(h, gi0):
            kt, qt = qkv_sb[(h, 1)], qkv_sb[(h, 0)]
            w = TW[h] * 128
            ps_kq = psum.tile([128, 512], F32, tag="kq", bufs=2, name="kqps")
            # additive -1e10 mask via identity matmul initializes the
            # PSUM accumulation; exp then yields exact zeros off-mask
            nc.tensor.matmul(
                ps_kq[:, :w], ident, mask_sb[:, MQOFF[h]:MQOFF[h] + w],
                start=True, stop=False)
            for q in range(TW[h]):
                c0 = (gi0 + q) * 128
                nc.tensor.matmul(
                    ps_kq[:, q * 128:(q + 1) * 128],
                    kt[:, c0:c0 + 128], qt[:, c0:c0 + 128],
                    start=False, stop=True)
            return ps_kq

        def emit_sm(h, gi0, ps_kq):
            tw = TW[h]
            w = tw * 128
            enumer = sm_pool.tile([128, 512], BF16, tag="enumer")
            nc.scalar.activation(enumer[:, :w], ps_kq[:, :w], EXP, scale=NORM)
            sums = small.tile([128, 4], F32, tag="sums")
            nc.vector.reduce_sum(
                sums[:, :tw],
                enumer[:, :w].rearrange("p (q c) -> p q c", c=128), axis=AX.X)
            recip = small.tile([128, 4], F32, tag="recip")
            nc.vector.reciprocal(recip[:, :tw], sums[:, :tw])
            smkq = sm_pool.tile([128, 512], BF16, tag="smkq")
            rc_bc = bass.AP(tensor=recip.tensor, offset=recip.offset,
                            ap=[recip.ap[0], [1, tw], [0, 128]])
            nc.gpsimd.tensor_mul(smkq[:, :w], enumer[:, :w], rc_bc)
            return smkq

        def emit_att(h, gi0, smkq):
            g, sl, tw = G[h], LS[h], TW[h]
            w = tw * 128
            ps_at = psum.tile([128, 512], F32, tag="at", bufs=2, name="atps")
            for q in range(tw):
                nc.tensor.matmul(ps_at[:, q * 128:(q + 1) * 128],
                                 vnat[h][:, gi0 + q, :],
                                 smkq[:, q * 128:(q + 1) * 128],
                                 start=True, stop=True)
            # single contiguous copy: psum cols (q,j,m) == s-major (s,m)
            s0 = gi0 * g
            rr_copy(atT[h][:, s0 * sl:s0 * sl + w], ps_at[:, :w])

        def emit_unit(cid, s0):
            ol, heads = CLASSES[cid]
            n_o = len(ol)
            sc = 128 // n_o
            ps_h = [psum.tile([128, 512], F32, tag="y", bufs=2,
                              name=f"psy{half}") for half in range(2)]
            for hi, h in enumerate(heads):
                ll = [o // DILS[h] for o in ol]
                dl = ll[1] - ll[0] if n_o > 1 else 1
                lhsT = atT[h].rearrange("p (s l) -> p s l", l=LS[h])[
                    :, s0:s0 + sc, ll[0]:ll[-1] + 1:dl]
                for half in range(2):
                    cs = half * 512
                    nc.tensor.matmul(
                        ps_h[half], lhsT,
                        wout_sb[:, h * E + cs:h * E + cs + 512],
                        start=(hi == 0), stop=(hi == len(heads) - 1))
            y_sb = yo_pool.tile([128, E], BF16, tag="ysb")
            rr_copy(y_sb[:, 0:512], ps_h[0])
            rr_copy(y_sb[:, 512:1024], ps_h[1])
            do = ol[1] - ol[0] if n_o > 1 else 1
            dst = y.rearrange("(s o) e -> s o e", o=64)[
                s0:s0 + sc, ol[0]:ol[-1] + 1:do, :]
            nc.sync.dma_start(out=dst, in_=y_sb)

        # ------------- merged schedule ------------------------------------
        # the DMA-paced start interleaves h0+h1 projection per x block
        # (16.2us of PE work per 2MB block > the ~12us block arrival gap);
        # attention tiles + ready out-proj units then weave into the
        # remaining h2/h3 projection streams so the PE-dense projection
        # hides every softmax-chain stall.  h0's last two attention tiles
        # run in the final window so their units (needing all heads) drain
        # there.
        # h0/h1/h2 projections run block-by-block as x streams in (the h0
        # l<8 tiles are gated only on the block's first half); h3's tiles
        # stay back as window padding.  x arrives slowly (a fixed runtime
        # DMA burst competes for bandwidth in the first ~40us), so front-
        # loading per-block work here minimizes the drought holes.
        for blk in range(NBLK):
            for p in range(3):
                proj_tile(0, p, blk, 0, 8)()
            for p in range(3):
                proj_tile(0, p, blk, 8, 8)()
            for c in proj_feed_blk(1, blk) + proj_feed_blk(2, blk):
                c()
            if blk == 0:
                # x block 1 lags block 0's work: keep the PE clock hot
                wps = psum.tile([128, 512], BF16, tag="pj", bufs=2,
                                name="warmfill")
                for _ in range(40):
                    nc.tensor.transpose(wps[:, 0:128], ident, ident)

        win_tiles = [
            [(0, g) for g in (0, 4, 8, 12, 16, 20)],
            [(1, g) for g in (0, 4, 8, 12)],
            [(2, g) for g in (0, 2, 4, 6)],
            [(0, 24), (0, 28)] + [(3, g) for g in (0, 1, 2, 3)],
        ]
        win_feed = [
            vt_feed(0),
            vt_feed(1) + proj_feed_blk(3, 0) + proj_feed_blk(3, 1),
            vt_feed(2) + proj_feed_blk(3, 2) + proj_feed_blk(3, 3),
            vt_feed(3),
        ]

        # out-proj unit (cid, s0) is eligible once every head in its class
        # has attended the s-range [s0, s0+sc); emit one step delayed
        s_done = {h: [False] * NS for h in range(H)}
        units = [(cid, s0) for cid in range(len(CLASSES))
                 for s0 in range(0, NS, 128 // len(CLASSES[cid][0]))]

        def take_eligible():
            out, rest = [], []
            for cid, s0 in units:
                sc = 128 // len(CLASSES[cid][0])
                if all(s_done[h][s] for h in CLASSES[cid][1]
                       for s in range(s0, s0 + sc)):
                    out.append((cid, s0))
                else:
                    rest.append((cid, s0))
            units[:] = rest
            return out

        # units are capped at 1 per step so the copy engines never crunch
        # mid-window; the backlog drains PE-dense after the last window
        pend = []
        for wi in range(4):
            tiles = win_tiles[wi]
            feed = win_feed[wi]
            nt = len(tiles)
            nsteps = nt + 2
            fed = 0
            kqs, sms = {}, {}
            for j in range(nsteps):
                # spread the feed evenly over this window's steps
                want = (len(feed) * (j + 1) + nsteps - 1) // nsteps
                while fed < want:
                    feed[fed]()
                    fed += 1
                if j < nt:
                    h, gi0 = tiles[j]
                    kqs[j] = emit_kq(h, gi0)
                if 0 <= j - 1 < nt:
                    h, gi0 = tiles[j - 1]
                    sms[j - 1] = emit_sm(h, gi0, kqs.pop(j - 1))
                for u in pend:
                    emit_unit(*u)
                pend = []
                if 0 <= j - 2 < nt:
                    h, gi0 = tiles[j - 2]
                    emit_att(h, gi0, sms.pop(j - 2))
                    g = G[h]
                    for s in range(gi0 * g, (gi0 + TW[h]) * g):
                        s_done[h][s] = True
                    pend.extend(take_eligible())
        for u in pend:
            emit_unit(*u)
        assert not units, f"unemitted units: {units}"


_NC = None


def _get_program():
    global _NC
    if _NC is None:
        _NC = build_program()
    return _NC


def _host_inputs(Wk, Wq, Wv, W_out, b_out):
    bf = ml_dtypes.bfloat16
    Wstack = np.stack([Wq, Wk, Wv], 1)                     # [H, 3, 128, 1024]
    tmp = Wstack.reshape(H, 3, 128, NECHUNK, 128)          # [h, p, c, ec, r]
    wqkv_sb = np.ascontiguousarray(
        tmp.transpose(0, 1, 4, 3, 2)).reshape(12, 128, NECHUNK * 128
                                              ).astype(bf)
    wout_sb = np.ascontiguousarray(
        W_out.reshape(E, H, 128).transpose(2, 1, 0)).reshape(128, H * E
                                                             ).astype(bf)
    # stacked additive mask per head, TW[h] copies of the [128, 128] base:
    # row p = k*L + n, col c = j*L + m; keep (0.0) iff j == k and m <= n
    # (softmax runs over the query axis m), else -1e10
    mask_host = np.full((128, MTOT), NEG, np.float32)
    for h in range(H):
        sl = LS[h]
        base = np.full((128, 128), NEG, np.float32)
        for p in range(128):
            k, nn = p // sl, p % sl
            base[p, k * sl:k * sl + nn + 1] = 0.0
        for q in range(TW[h]):
            c0 = MQOFF[h] + q * 128
            mask_host[:, c0:c0 + 128] = base
    return wqkv_sb, wout_sb, mask_host.astype(bf)


def _shard_x(xbf, c):
    b, half = c // 2, c % 2
    xs = xbf[b].reshape(NB, SEG, E)[:, half * NS:(half + 1) * NS, :]
    xs = xs.reshape(ROWS, E)                       # rows (n, s)
    # device layout: [e-chunk, blk, e-in-chunk, row] (x^T per 128-wide e chunk)
    return np.ascontiguousarray(
        xs.reshape(NBLK, BLKN * NS, NECHUNK, 128).transpose(2, 0, 3, 1))


def _prepare(x, Wk, Wq, Wv, W_out, b_out):
    xbf = np.asarray(x, np.float32).astype(ml_dtypes.bfloat16)
    wqkv_sb, wout_sb, mask_host = _host_inputs(
        np.asarray(Wk, np.float32), np.asarray(Wq, np.float32),
        np.asarray(Wv, np.float32), np.asarray(W_out, np.float32),
        np.asarray(b_out, np.float32))
    in_maps = []
    for c in range(8):
        in_maps.append({"xs": _shard_x(xbf, c), "wqkv": wqkv_sb,
                        "wout": wout_sb, "masks": mask_host})
    return _get_program(), in_maps


def _gather(res, b_out):
    y = np.empty((B, T, E), np.float32)
    for c in range(8):
        b, half = c // 2, c % 2
        y[b, half * ROWS:(half + 1) * ROWS, :] = \
            res.results[c]["y"].astype(np.float32)
    y += np.asarray(b_out, np.float32).reshape(1, 1, E)
    return y


def kernel(x, Wk, Wq, Wv, W_out, b_out):
    nc, in_maps = _prepare(x, Wk, Wq, Wv, W_out, b_out)
    res = run_bass_kernel_spmd(nc, in_maps, core_ids=list(range(8)))
    return _gather(res, b_out)


# revision 43
# speedup vs baseline: 1.1203x; 1.1203x over previous
"""Trainium2 Bass kernel for MultiHeadDilatedAttention.

Full inputs in, full output out. Sharding: 8 cores = (batch b in 0..3) x
(segment-position half). Each (b, s) pair is an independent attention problem
(attention runs across segments n at fixed position-in-segment s), so each
core handles b = c//2 and 64 of the 128 s values. No collectives needed: the
output rows t = s*64 + dil*l for a core's s-range form a contiguous chunk of
y[b].

v3 layout (all matmuls bf16 with fp32 PSUM accumulation):
  x cast to bf16 + transposed on host -> [ec, blk, 128, 1024] in DRAM
  pipelined per 16-segment block: DMA x block, project Q^T/K^T/V^T
  Q/K/V stored s-major fully packed (slot stride = L per head, no padding)
  V^T -> PE-transpose -> V natural [slot, dv] per 128-col group
  attention phase is s-block-major (16 s per block, all 4 heads), 2-deep
  software pipeline; masking is a 0/1 bf16 multiply on GpSimd after the
  Scalar exp (no PE mask-init matmuls); out-projection units for block sb
  are interleaved into block sb+1's attention steps so the PE never idles
  during the softmax chain. y stored bf16, upcast + b_out added on host.
"""

from contextlib import ExitStack

import numpy as np
import ml_dtypes

import concourse.bass as bass
import concourse.mybir as mybir
import concourse.tile as tile
from concourse import bacc
from concourse.bass_utils import run_bass_kernel_spmd

F32 = mybir.dt.float32
BF16 = mybir.dt.bfloat16
AX = mybir.AxisListType
EXP = mybir.ActivationFunctionType.Exp

B, T, E = 4, 8192, 1024
SEG = 128          # segment size (= #s positions overall)
NB = T // SEG      # 64 segments
NS = 64            # s values per core
ROWS = NB * NS     # 4096 rows per core
DK = 128
H = 4
DILS = [1, 2, 4, 8]
LS = [NB // d for d in DILS]       # [64, 32, 16, 8] attention length per head
G = [128 // l for l in LS]         # s-slots per 128 partitions: [2, 4, 8, 16]
NGRP = [NS // g for g in G]        # 128-col groups per head: [32, 16, 8, 4]
TW = [4, 4, 2, 1]                  # groups per attention tile (tile = 16 or 8 s)
NORM = float(1.0 / np.sqrt(DK))
NECHUNK = E // 128                 # 8
NBLK = 4                           # x pipeline blocks
BLKN = NB // NBLK                  # 16 segments per block
SBLK = 4                           # attention s-blocks (16 s each)
SBW = NS // SBLK                   # 16
MQOFF = [0, 512, 1024, 1280]       # mask col offset per head (stacked masks)
MTOT = 1408                        # sum of TW[h]*128
NEG = -1.0e10

# out-projection classes: offsets o in [0,64) grouped by which heads hit them.
def _classes():
    out = []
    o_all = list(range(64))
    out.append(([o for o in o_all if o % 2 == 1], [0]))           # odd
    out.append(([o for o in o_all if o % 4 == 2], [0, 1]))        # 2 mod 4
    out.append(([o for o in o_all if o % 8 == 4], [0, 1, 2]))     # 4 mod 8
    out.append(([o for o in o_all if o % 8 == 0], [0, 1, 2, 3]))  # 0 mod 8
    return out


CLASSES = _classes()
# atT[h] column layout: per-class blocks (so out-proj lhsT slices are
# contiguous). HEAD_BLOCKS[h] = [(cid, l_list)], HEAD_OFF[h][cid] = col offset.
HEAD_BLOCKS = {}
HEAD_OFF = {}
for _h in range(H):
    blocks, offs, off = [], {}, 0
    for _cid, (_ol, _heads) in enumerate(CLASSES):
        if _h in _heads:
            ll = [o // DILS[_h] for o in _ol]
            blocks.append((_cid, ll))
            offs[_cid] = off
            off += len(ll) * NS
    HEAD_BLOCKS[_h] = blocks
    HEAD_OFF[_h] = offs
    assert off == LS[_h] * NS


def build_program(max_phase: int = 5) -> bass.Bass:
    nc = bacc.Bacc("TRN2", target_bir_lowering=False, debug=False)
    xs = nc.dram_tensor("xs", [NECHUNK, NBLK, 128, BLKN * NS], BF16,
                        kind="ExternalInput").ap()
    wqkv = nc.dram_tensor("wqkv", [12, 128, NECHUNK * 128], BF16,
                          kind="ExternalInput").ap()
    wout = nc.dram_tensor("wout", [128, H * E], BF16, kind="ExternalInput").ap()
    maskd = nc.dram_tensor("masks", [128, MTOT], BF16,
                           kind="ExternalInput").ap()
    y = nc.dram_tensor("y", [ROWS, E], BF16, kind="ExternalOutput").ap()
    _build_phases(nc, max_phase, xs, wqkv, wout, maskd, y)
    nc.finalize()
    return nc


def _build_phases(nc, max_phase, xs, wqkv, wout, maskd, y):
    # round-robin engine copy (PSUM->SBUF traffic spread over DVE + Act;
    # GPSIMD cannot access PSUM)
    state = {"i": 0}
    _RR = (0, 0, 0, 1, 1)  # DVE is ~1.6x faster per copy: give it 60%

    def rr_copy(out_ap, in_ap):
        k = _RR[state["i"] % len(_RR)]
        state["i"] += 1
        if k == 0:
            nc.vector.tensor_copy(out=out_ap, in_=in_ap)
        else:
            nc.scalar.copy(out_ap, in_ap)

    with ExitStack() as ctx:
        tc = ctx.enter_context(tile.TileContext(nc))

        persist = ctx.enter_context(tc.tile_pool(name="persist", bufs=1))
        ident = persist.tile([128, 128], BF16, tag="ident")
        # identity built before any dma_start posts: affine_select lives on
        # gpsimd, which also issues DMA descriptors — ident must come first
        # so the PE warm-up transposes can start at ~0.3us
        nc.gpsimd.memset(ident, 0.0)
        nc.gpsimd.affine_select(
            out=ident, in_=ident, compare_op=mybir.AluOpType.not_equal,
            fill=1.0, base=0, pattern=[[-1, 128]], channel_multiplier=1)

        # all inputs staged up front (x fully resident: 8 MiB of SBUF);
        # descriptor posts round-robin over sync/scalar/gpsimd, ordered so
        # the h0 weights + x block 0 land first
        w_sb = {}
        for h in range(H):
            for p in range(3):
                w_sb[(h, p)] = persist.tile([128, NECHUNK * 128], BF16,
                                            tag=f"w{h}{p}", name=f"w{h}{p}")
        xt_pool = ctx.enter_context(tc.tile_pool(name="xt", bufs=1))
        xt = {}
        for blk in range(NBLK):
            for ec in range(NECHUNK):
                xt[(blk, ec)] = xt_pool.tile(
                    [128, BLKN * NS], BF16, tag=f"x{blk}_{ec}",
                    name=f"x{blk}_{ec}")
        wout_sb = persist.tile([128, H * E], BF16, tag="wout_sb")
        mask_sb = persist.tile([128, MTOT], BF16, tag="mask_sb")

        # x tiles load in two 512-col halves; all blocks' first halves load
        # first so h0's l<8 projection tiles (which only read cols 0:512)
        # unlock ~21us of PE work before the second halves arrive — this
        # hides the x-DMA latency (DMA shares bandwidth with i-fetch)
        posts = [(w_sb[(0, 0)], wqkv[0, :, :])]
        for blk in range(NBLK):
            for ec in range(NECHUNK):
                posts.append((xt[(blk, ec)], xs[ec, blk, :, :]))
            if blk == 0:
                posts.append((w_sb[(0, 1)], wqkv[1, :, :]))
                posts.append((w_sb[(0, 2)], wqkv[2, :, :]))
        for h in (1, 2, 3):
            for p in range(3):
                posts.append((w_sb[(h, p)], wqkv[h * 3 + p, :, :]))
        posts.append((mask_sb, maskd))
        posts.append((wout_sb, wout))
        dma_engs = [nc.sync, nc.scalar, nc.gpsimd]
        for i, (dst, src) in enumerate(posts):
            dma_engs[i % 3].dma_start(out=dst, in_=src)

        # warm the PE clock gate (HAM) during the initial DMA wait: ~3.5us of
        # dummy transposes flips the PE to 2.4 GHz before real work arrives
        with ExitStack() as wctx:
            warm_ps = wctx.enter_context(
                tc.tile_pool(name="warm_ps", bufs=1, space="PSUM"))
            wps = warm_ps.tile([128, 128], BF16, tag="warm")
            for _ in range(36):
                nc.tensor.transpose(wps, ident, ident)

        # persistent per-head tensors: Q^T/K^T/V^T all stored s-major fully
        # packed (col = s*L + l), so a slot-group slice is one contiguous
        # 128-col window with no dead slots.
        qkvpool = ctx.enter_context(tc.tile_pool(name="qkv", bufs=1))
        qkv_sb = {}
        for h in range(H):
            for p in range(3):
                qkv_sb[(h, p)] = qkvpool.tile([128, LS[h] * NS], BF16,
                                              tag=f"qkv{h}{p}", name=f"qkv{h}{p}")
        atpool = ctx.enter_context(tc.tile_pool(name="atT", bufs=1))
        # plain s-major layout (col = s*L + l): the att scatter is then ONE
        # contiguous copy per tile; the out-proj reads strided lhsT slices
        # instead (stride cost moves to LDWEIGHTS, off the copy engines)
        atT = [atpool.tile([128, LS[h] * NS], BF16, tag=f"atT{h}",
                           name=f"atT{h}") for h in range(H)]
        vnpool = ctx.enter_context(tc.tile_pool(name="vnat", bufs=1))
        # vnat[h]: [128, NGRP, 128]; group gi holds V for s = gi*G[h]..+G[h]
        # at partition slots (s % G[h]) * L[h] + n
        vnat = [vnpool.tile([128, NGRP[h], 128], BF16, tag=f"vnat{h}",
                            name=f"vnat{h}") for h in range(H)]

        # one PSUM pool, tag = bank budget: pj 2 + kq 2 + at 1 + y 3 = 8
        psum = ctx.enter_context(tc.tile_pool(name="psum", bufs=1,
                                              space="PSUM"))
        sm_pool = ctx.enter_context(tc.tile_pool(name="sm", bufs=3))
        small = ctx.enter_context(tc.tile_pool(name="small", bufs=4))
        yo_pool = ctx.enter_context(tc.tile_pool(name="y_sb", bufs=3))

        # ---------------- emission closures -------------------------------
        def proj_tile(h, p, blk, lt, lcnt):
            def emit():
                dil = DILS[h]
                tcols = lcnt * NS
                l0 = blk * (BLKN // dil)
                ps = psum.tile([128, 512], F32, tag="pj", bufs=2,
                               name="pjps")[:, :tcols]
                for ec in range(NECHUNK):
                    lhsT = w_sb[(h, p)][:, ec * 128:(ec + 1) * 128]
                    rhs = xt[(blk, ec)].rearrange(
                        "p (l j s) -> p l j s", j=dil, s=NS
                    )[:, lt:lt + lcnt, 0, :]
                    nc.tensor.matmul(ps, lhsT, rhs, start=(ec == 0),
                                     stop=(ec == NECHUNK - 1))
                out_ap = qkv_sb[(h, p)].rearrange(
                    "p (s l) -> p s l", l=LS[h]
                )[:, :, l0 + lt:l0 + lt + lcnt]
                rr_copy(out_ap, ps.rearrange("p (l s) -> p s l", s=NS))
            return emit

        def proj_feed_blk(h, blk):
            out = []
            ln = BLKN // DILS[h]
            for p in range(3):
                for lt in range(0, ln, 8):
                    out.append(proj_tile(h, p, blk, lt, min(8, ln - lt)))
            return out

        def vt_batch(h, gi):
            def emit():
                # 4 transposes into one psum tile -> one wide PSUM->SBUF copy
                vt = qkv_sb[(h, 2)]
                pt = psum.tile([128, 512], BF16, tag="pj", bufs=2, name="vtps")
                for q in range(4):
                    nc.tensor.transpose(
                        pt[:, q * 128:(q + 1) * 128],
                        vt[:, (gi + q) * 128:(gi + q + 1) * 128], ident)
                rr_copy(vnat[h][:, gi:gi + 4, :], pt)
            return emit

        def vt_feed(h):
            return [vt_batch(h, gi) for gi in range(0, NGRP[h], 4)]

        def emit_kq(h, gi0):
            kt, qt = qkv_sb[(h, 1)], qkv_sb[(h, 0)]
            w = TW[h] * 128
            ps_kq = psum.tile([128, 512], F32, tag="kq", bufs=2, name="kqps")
            # additive -1e10 mask via identity matmul initializes the
            # PSUM accumulation; exp then yields exact zeros off-mask
            nc.tensor.matmul(
                ps_kq[:, :w], ident, mask_sb[:, MQOFF[h]:MQOFF[h] + w],
                start=True, stop=False)
            for q in range(TW[h]):
                c0 = (gi0 + q) * 128
                nc.tensor.matmul(
                    ps_kq[:, q * 128:(q + 1) * 128],
                    kt[:, c0:c0 + 128], qt[:, c0:c0 + 128],
                    start=False, stop=True)
            return ps_kq

        def emit_sm(h, gi0, ps_kq):
            tw = TW[h]
            w = tw * 128
            enumer = sm_pool.tile([128, 512], BF16, tag="enumer")
            nc.scalar.activation(enumer[:, :w], ps_kq[:, :w], EXP, scale=NORM)
            sums = small.tile([128, 4], F32, tag="sums")
            nc.vector.reduce_sum(
                sums[:, :tw],
                enumer[:, :w].rearrange("p (q c) -> p q c", c=128), axis=AX.X)
            recip = small.tile([128, 4], F32, tag="recip")
            nc.vector.reciprocal(recip[:, :tw], sums[:, :tw])
            smkq = sm_pool.tile([128, 512], BF16, tag="smkq")
            rc_bc = bass.AP(tensor=recip.tensor, offset=recip.offset,
                            ap=[recip.ap[0], [1, tw], [0, 128]])
            nc.gpsimd.tensor_mul(smkq[:, :w], enumer[:, :w], rc_bc)
            return smkq

        def emit_att(h, gi0, smkq):
            g, sl, tw = G[h], LS[h], TW[h]
            w = tw * 128
            ps_at = psum.tile([128, 512], F32, tag="at", bufs=1, name="atps")
            for q in range(tw):
                nc.tensor.matmul(ps_at[:, q * 128:(q + 1) * 128],
                                 vnat[h][:, gi0 + q, :],
                                 smkq[:, q * 128:(q + 1) * 128],
                                 start=True, stop=True)
            # single contiguous copy: psum cols (q,j,m) == s-major (s,m)
            s0 = gi0 * g
            rr_copy(atT[h][:, s0 * sl:s0 * sl + w], ps_at[:, :w])

        def emit_unit(cid, s0):
            ol, heads = CLASSES[cid]
            n_o = len(ol)
            sc = 128 // n_o
            ps_h = [psum.tile([128, 512], F32, tag="y", bufs=3,
                              name=f"psy{half}") for half in range(2)]
            for hi, h in enumerate(heads):
                ll = [o // DILS[h] for o in ol]
                dl = ll[1] - ll[0] if n_o > 1 else 1
                lhsT = atT[h].rearrange("p (s l) -> p s l", l=LS[h])[
                    :, s0:s0 + sc, ll[0]:ll[-1] + 1:dl]
                for half in range(2):
                    cs = half * 512
                    nc.tensor.matmul(
                        ps_h[half], lhsT,
                        wout_sb[:, h * E + cs:h * E + cs + 512],
                        start=(hi == 0), stop=(hi == len(heads) - 1))
            y_sb = yo_pool.tile([128, E], BF16, tag="ysb")
            rr_copy(y_sb[:, 0:512], ps_h[0])
            rr_copy(y_sb[:, 512:1024], ps_h[1])
            do = ol[1] - ol[0] if n_o > 1 else 1
            dst = y.rearrange("(s o) e -> s o e", o=64)[
                s0:s0 + sc, ol[0]:ol[-1] + 1:do, :]
            nc.sync.dma_start(out=dst, in_=y_sb)

        # ------------- merged schedule ------------------------------------
        # the DMA-paced start interleaves h0+h1 projection per x block
        # (16.2us of PE work per 2MB block > the ~12us block arrival gap);
        # attention tiles + ready out-proj units then weave into the
        # remaining h2/h3 projection streams so the PE-dense projection
        # hides every softmax-chain stall.  h0's last two attention tiles
        # run in the final window so their units (needing all heads) drain
        # there.
        # h0/h1/h2 projections run block-by-block as x streams in (the h0
        # l<8 tiles are gated only on the block's first half); h3's tiles
        # stay back as window padding.  x arrives slowly (a fixed runtime
        # DMA burst competes for bandwidth in the first ~40us), so front-
        # loading per-block work here minimizes the drought holes.
        for blk in range(NBLK):
            for p in range(3):
                proj_tile(0, p, blk, 0, 8)()
            for p in range(3):
                proj_tile(0, p, blk, 8, 8)()
            for c in proj_feed_blk(1, blk):
                c()
            if blk == 0:
                # x block 1 lags block 0's work: keep the PE clock hot
                wps = psum.tile([128, 512], BF16, tag="pj", bufs=2,
                                name="warmfill")
                for _ in range(40):
                    nc.tensor.transpose(wps[:, 0:128], ident, ident)

        win_tiles = [
            [(0, g) for g in (0, 4, 8, 12, 16, 20)],
            [(1, g) for g in (0, 4, 8, 12)],
            [(2, g) for g in (0, 2, 4, 6)],
            [(0, 24), (0, 28)] + [(3, g) for g in (0, 1, 2, 3)],
        ]
        win_feed = [
            vt_feed(0) + [c for b in range(NBLK) for c in proj_feed_blk(2, b)],
            vt_feed(1) + proj_feed_blk(3, 0) + proj_feed_blk(3, 1),
            vt_feed(2) + proj_feed_blk(3, 2) + proj_feed_blk(3, 3),
            vt_feed(3),
        ]

        # out-proj unit (cid, s0) is eligible once every head in its class
        # has attended the s-range [s0, s0+sc); emit one step delayed
        s_done = {h: [False] * NS for h in range(H)}
        units = [(cid, s0) for cid in range(len(CLASSES))
                 for s0 in range(0, NS, 128 // len(CLASSES[cid][0]))]

        def take_eligible():
            out, rest = [], []
            for cid, s0 in units:
                sc = 128 // len(CLASSES[cid][0])
                if all(s_done[h][s] for h in CLASSES[cid][1]
                       for s in range(s0, s0 + sc)):
                    out.append((cid, s0))
                else:
                    rest.append((cid, s0))
            units[:] = rest
            return out

        # units are capped at 1 per step so the copy engines never crunch
        # mid-window; the backlog drains PE-dense after the last window
        pend = []
        for wi in range(4):
            tiles = win_tiles[wi]
            feed = win_feed[wi]
            nt = len(tiles)
            nsteps = nt + 2
            fed = 0
            kqs, sms = {}, {}
            for j in range(nsteps):
                # spread the feed evenly over this window's steps
                want = (len(feed) * (j + 1) + nsteps - 1) // nsteps
                while fed < want:
                    feed[fed]()
                    fed += 1
                if j < nt:
                    h, gi0 = tiles[j]
                    kqs[j] = emit_kq(h, gi0)
                if 0 <= j - 1 < nt:
                    h, gi0 = tiles[j - 1]
                    sms[j - 1] = emit_sm(h, gi0, kqs.pop(j - 1))
                if pend:
                    emit_unit(*pend.pop(0))
                if 0 <= j - 2 < nt:
                    h, gi0 = tiles[j - 2]
                    emit_att(h, gi0, sms.pop(j - 2))
                    g = G[h]
                    for s in range(gi0 * g, (gi0 + TW[h]) * g):
                        s_done[h][s] = True
                    pend.extend(take_eligible())
        for u in pend:
            emit_unit(*u)
        assert not units, f"unemitted units: {units}"


_NC = None


def _get_program():
    global _NC
    if _NC is None:
        _NC = build_program()
    return _NC


def _host_inputs(Wk, Wq, Wv, W_out, b_out):
    bf = ml_dtypes.bfloat16
    Wstack = np.stack([Wq, Wk, Wv], 1)                     # [H, 3, 128, 1024]
    tmp = Wstack.reshape(H, 3, 128, NECHUNK, 128)          # [h, p, c, ec, r]
    wqkv_sb = np.ascontiguousarray(
        tmp.transpose(0, 1, 4, 3, 2)).reshape(12, 128, NECHUNK * 128
                                              ).astype(bf)
    wout_sb = np.ascontiguousarray(
        W_out.reshape(E, H, 128).transpose(2, 1, 0)).reshape(128, H * E
                                                             ).astype(bf)
    # stacked additive mask per head, TW[h] copies of the [128, 128] base:
    # row p = k*L + n, col c = j*L + m; keep (0.0) iff j == k and m <= n
    # (softmax runs over the query axis m), else -1e10
    mask_host = np.full((128, MTOT), NEG, np.float32)
    for h in range(H):
        sl = LS[h]
        base = np.full((128, 128), NEG, np.float32)
        for p in range(128):
            k, nn = p // sl, p % sl
            base[p, k * sl:k * sl + nn + 1] = 0.0
        for q in range(TW[h]):
            c0 = MQOFF[h] + q * 128
            mask_host[:, c0:c0 + 128] = base
    return wqkv_sb, wout_sb, mask_host.astype(bf)


def _shard_x(xbf, c):
    b, half = c // 2, c % 2
    xs = xbf[b].reshape(NB, SEG, E)[:, half * NS:(half + 1) * NS, :]
    xs = xs.reshape(ROWS, E)                       # rows (n, s)
    # device layout: [e-chunk, blk, e-in-chunk, row] (x^T per 128-wide e chunk)
    return np.ascontiguousarray(
        xs.reshape(NBLK, BLKN * NS, NECHUNK, 128).transpose(2, 0, 3, 1))


def _prepare(x, Wk, Wq, Wv, W_out, b_out):
    xbf = np.asarray(x, np.float32).astype(ml_dtypes.bfloat16)
    wqkv_sb, wout_sb, mask_host = _host_inputs(
        np.asarray(Wk, np.float32), np.asarray(Wq, np.float32),
        np.asarray(Wv, np.float32), np.asarray(W_out, np.float32),
        np.asarray(b_out, np.float32))
    in_maps = []
    for c in range(8):
        in_maps.append({"xs": _shard_x(xbf, c), "wqkv": wqkv_sb,
                        "wout": wout_sb, "masks": mask_host})
    return _get_program(), in_maps


def _gather(res, b_out):
    y = np.empty((B, T, E), np.float32)
    for c in range(8):
        b, half = c // 2, c % 2
        y[b, half * ROWS:(half + 1) * ROWS, :] = \
            res.results[c]["y"].astype(np.float32)
    y += np.asarray(b_out, np.float32).reshape(1, 1, E)
    return y


def kernel(x, Wk, Wq, Wv, W_out, b_out):
    nc, in_maps = _prepare(x, Wk, Wq, Wv, W_out, b_out)
    res = run_bass_kernel_spmd(nc, in_maps, core_ids=list(range(8)))
    return _gather(res, b_out)
